# revision 21
# baseline (speedup 1.0000x reference)
"""Trainium2 Bass kernel for the attention module:

    att_h  = h @ W_h2att.T + b_h2att             # [B, 512]
    dot    = tanh(p_att_feats + att_h[:, None])  # [B, 1024, 512]
    scores = dot @ w_alpha + b_alpha             # [B, 1024]
    weight = softmax(scores, axis=1)
    out    = einsum('bs,bsd->bd', weight, att_feats)  # [B, 2048]

Sharding: data-parallel over batch B=64 across 8 NeuronCores (8 per core).
Params are tiny and replicated. b_alpha is a softmax shift -> dropped.

Per-core structure (b-major pipeline, all fp32):
  setup: att_h via TensorE (host-transposed W.T/h.T), broadcast rows via
         DRAM bounce + partition_broadcast
  per b: score tiles (DVE add + ScalarE tanh + DVE mul/reduce)
         -> per-b softmax in [t, s-in-tile] layout (TensorE transposes for
            partition reductions, exp accum_out for the denominator)
         -> unnormalized weighted sum via M=1 PSUM-accumulating matmuls
         -> normalize the [1, 2048] row by 1/Z, DMA out
"""

import numpy as np

import concourse.bass as bass
import concourse.tile as tile
from concourse import bacc, mybir
from concourse.bass import ts
from concourse.bass_utils import run_bass_kernel_spmd

F32 = mybir.dt.float32
F32R = mybir.dt.float32r

B_LOC = 8       # batches per core
S = 1024        # attended positions
ST = S // 128   # 8 s-tiles of 128
G = 2           # s-tiles per DMA group
NG = ST // G    # 4 groups
HID = 512
D = 2048
DT = D // 512   # 4 output column slices
K = 2048        # rnn_size (contraction for att_h)
KT = K // 128   # 16 k-tiles

_NC_CACHE = None


def build_kernel(att_bufs=8):
    nc = bacc.Bacc("TRN2", target_bir_lowering=False, debug=False, num_devices=8)

    p_d = nc.dram_tensor("p", [B_LOC, S, HID], F32, kind="ExternalInput")
    att_d = nc.dram_tensor("att", [B_LOC, S, D], F32, kind="ExternalInput")
    hT_d = nc.dram_tensor("hT", [K, B_LOC], F32, kind="ExternalInput")
    WT_d = nc.dram_tensor("WT", [K, HID], F32, kind="ExternalInput")
    wab_d = nc.dram_tensor("wab", [128, HID], F32, kind="ExternalInput")
    bias8_d = nc.dram_tensor("bias8", [B_LOC, HID], F32, kind="ExternalInput")
    ident_d = nc.dram_tensor("ident", [128, 128], F32, kind="ExternalInput")
    out_d = nc.dram_tensor("out", [B_LOC, D], F32, kind="ExternalOutput")
    scratch_d = nc.dram_tensor("atth_scratch", [B_LOC, HID], F32)

    with tile.TileContext(nc) as tc:
        with (
            tc.tile_pool(name="consts", bufs=1) as consts,
            tc.tile_pool(name="singles", bufs=1) as singles,
            tc.tile_pool(name="wt", bufs=2) as wtpool,
            tc.tile_pool(name="ht", bufs=3) as htpool,
            tc.tile_pool(name="ahbc", bufs=B_LOC) as ahbcpool,
            tc.tile_pool(name="pp", bufs=3) as ppool,
            tc.tile_pool(name="th", bufs=3) as thpool,
            tc.tile_pool(name="sct", bufs=2) as sctpool,
            tc.tile_pool(name="small", bufs=2) as smallpool,
            tc.tile_pool(name="wgtp", bufs=2) as wgtpool,
            tc.tile_pool(name="attp", bufs=att_bufs) as attpool,
            tc.tile_pool(name="rowp", bufs=2) as rowpool,
            tc.tile_pool(name="ps_setup", bufs=1, space=bass.MemorySpace.PSUM) as ps_setup,
            tc.tile_pool(name="ps_tp", bufs=3, space=bass.MemorySpace.PSUM) as ps_tp,
            tc.tile_pool(name="ps_acc", bufs=4, space=bass.MemorySpace.PSUM) as ps_acc,
        ):
            # ---- constants ----
            wab = consts.tile([128, HID], F32)
            nc.sync.dma_start(wab[:], wab_d[:])
            ident = consts.tile([128, 128], F32)
            nc.sync.dma_start(ident[:], ident_d[:])
            bias8 = consts.tile([B_LOC, HID], F32)
            nc.sync.dma_start(bias8[:], bias8_d[:])

            # ---- att_h = h @ W.T + b  ([8, 512]) ----
            atth_ps = ps_setup.tile([B_LOC, HID], F32)
            KJ = 2
            KG = KT // KJ
            WT_r = WT_d.rearrange("(kg q j) h -> kg q j h", q=128, j=KJ)
            hT_r = hT_d.rearrange("(kg q j) h -> kg q j h", q=128, j=KJ)
            for kg in range(KG):
                wt = wtpool.tile([128, KJ, HID], F32)
                nc.sync.dma_start(wt[:], WT_r[kg])
                ht = htpool.tile([128, KJ, B_LOC], F32)
                nc.sync.dma_start(ht[:], hT_r[kg])
                for j in range(KJ):
                    nc.tensor.matmul(
                        atth_ps[:], ht[:, j, :], wt[:, j, :],
                        start=(kg == 0 and j == 0),
                        stop=(kg == KG - 1 and j == KJ - 1),
                    )
            A = singles.tile([B_LOC, HID], F32)
            nc.scalar.copy(A[:], atth_ps[:])
            A2 = singles.tile([B_LOC, HID], F32)
            nc.vector.tensor_add(A2[:], A[:], bias8[:])

            # broadcast att_h rows across 128 partitions
            nc.sync.dma_start(scratch_d[:], A2[:])
            ahbc = []
            for b in range(B_LOC):
                row = rowpool.tile([1, HID], F32, name=f"ahrow{b}", tag="ahrow")
                nc.sync.dma_start(row[:], scratch_d[b : b + 1, :])
                t = ahbcpool.tile([128, HID], F32, name=f"ahbc{b}", tag="ahbc")
                nc.gpsimd.partition_broadcast(t[:], row[:])
                ahbc.append(t)

            p_r = [
                p_d[b].rearrange("(g q j) h -> g q j h", q=128, j=G)
                for b in range(B_LOC)
            ]
            att_r = [
                att_d[b].rearrange("(g q j) h -> g q j h", q=128, j=G)
                for b in range(B_LOC)
            ]

            wgtT = {}
            rzs = {}

            def emit_scores(b):
                # ---- scores for batch b: sc_b[s_in_tile, col] ----
                sc_b = sctpool.tile([128, ST], F32, name=f"sc{b}", tag="sc")
                for g in range(NG):
                    pt = ppool.tile([128, G, HID], F32, name=f"pt{b}_{g}", tag="pt")
                    nc.sync.dma_start(pt[:], p_r[b][g])
                    nc.vector.tensor_add(
                        pt[:], pt[:],
                        ahbc[b][:, None, :].broadcast_to((128, G, HID)),
                    )
                    th = thpool.tile([128, G, HID], F32, name=f"th{b}_{g}", tag="th")
                    nc.scalar.activation(
                        th[:], pt[:], mybir.ActivationFunctionType.Tanh
                    )
                    nc.vector.tensor_mul(
                        th[:], th[:],
                        wab[:, None, :].broadcast_to((128, G, HID)),
                    )
                    nc.vector.reduce_sum(
                        sc_b[:, ts(g, G)], th[:], axis=mybir.AxisListType.X
                    )

                # ---- per-b softmax (weights left unnormalized) ----
                tp1 = ps_tp.tile([ST, 128], F32, name=f"tp1_{b}", tag="tp")
                nc.tensor.transpose(tp1[:], sc_b[:], ident[:])
                Sb = smallpool.tile([ST, 128], F32, name=f"Sb{b}", tag="Sb")
                nc.scalar.copy(Sb[:], tp1[:])
                m8 = smallpool.tile([ST, 1], F32, name=f"m8{b}", tag="m8")
                nc.vector.reduce_max(m8[:], Sb[:], axis=mybir.AxisListType.X)
                tp2 = ps_tp.tile([1, ST], F32, name=f"tp2_{b}", tag="tp")
                nc.tensor.transpose(tp2[:], m8[:], ident[:ST, :ST])
                m1 = smallpool.tile([1, ST], F32, name=f"m1{b}", tag="m1")
                nc.scalar.copy(m1[:], tp2[:])
                gmneg = smallpool.tile([1, 1], F32, name=f"gm{b}", tag="gm")
                nc.vector.reduce_max(
                    gmneg[:], m1[:], axis=mybir.AxisListType.X, negate=True
                )
                gm8 = smallpool.tile([ST, 1], F32, name=f"gm8{b}", tag="gm8")
                nc.gpsimd.partition_broadcast(gm8[:], gmneg[:])
                Eb = smallpool.tile([ST, 128], F32, name=f"Eb{b}", tag="Eb")
                z8 = smallpool.tile([ST, 1], F32, name=f"z8{b}", tag="z8")
                nc.scalar.activation(
                    Eb[:], Sb[:], mybir.ActivationFunctionType.Exp,
                    bias=gm8[:], accum_out=z8[:],
                )
                tp3 = ps_tp.tile([1, ST], F32, name=f"tp3_{b}", tag="tp")
                nc.tensor.transpose(tp3[:], z8[:], ident[:ST, :ST])
                z1 = smallpool.tile([1, ST], F32, name=f"z1{b}", tag="z1")
                nc.scalar.copy(z1[:], tp3[:])
                Z = smallpool.tile([1, 1], F32, name=f"Z{b}", tag="Z")
                nc.vector.reduce_sum(Z[:], z1[:], axis=mybir.AxisListType.X)
                rz = smallpool.tile([1, 1], F32, name=f"rz{b}", tag="rz")
                nc.vector.reciprocal(rz[:], Z[:])
                rzs[b] = rz
                tp4 = ps_tp.tile([128, ST], F32, name=f"tp4_{b}", tag="tp")
                nc.tensor.transpose(tp4[:], Eb[:], ident[:ST, :ST])
                w_sb = wgtpool.tile([128, ST], F32R, name=f"wgtT{b}", tag="wgtT")
                nc.scalar.copy(w_sb[:], tp4[:])
                wgtT[b] = w_sb

            def emit_weighted(b):
                accs = [
                    ps_acc.tile([1, 512], F32, name=f"acc{b}_{d}", tag="acc")
                    for d in range(DT)
                ]
                for g in range(NG):
                    at = attpool.tile([128, G, D], F32R, name=f"at{b}_{g}", tag="at")
                    nc.sync.dma_start(at[:], att_r[b][g].bitcast(F32R))
                    for u in range(G):
                        t = g * G + u
                        for d in range(DT):
                            nc.tensor.matmul(
                                accs[d][:],
                                wgtT[b][:, t : t + 1],
                                at[:, u, ts(d, 512)],
                                start=(t == 0),
                                stop=(t == ST - 1),
                            )
                rowbuf = rowpool.tile([1, D], F32, name=f"row{b}", tag="rowbuf")
                for d in range(DT):
                    nc.scalar.copy(rowbuf[0:1, ts(d, 512)], accs[d][:])
                nc.vector.tensor_scalar_mul(rowbuf[:], rowbuf[:], rzs[b][:])
                nc.sync.dma_start(out_d[b : b + 1, :], rowbuf[:])

            emit_scores(0)
            for b in range(B_LOC):
                if b + 1 < B_LOC:
                    emit_scores(b + 1)
                emit_weighted(b)

    nc.compile()
    return nc


def _in_maps(h, att_feats, p_att_feats, W_h2att, b_h2att, w_alpha):
    WT = np.ascontiguousarray(W_h2att.T).astype(np.float32)
    wab = np.ascontiguousarray(
        np.broadcast_to(w_alpha.astype(np.float32), (128, HID))
    )
    bias8 = np.ascontiguousarray(
        np.broadcast_to(b_h2att.astype(np.float32), (B_LOC, HID))
    )
    ident = np.eye(128, dtype=np.float32)
    maps = []
    for c in range(8):
        sl = slice(c * B_LOC, (c + 1) * B_LOC)
        maps.append(
            {
                "p": np.ascontiguousarray(p_att_feats[sl]).astype(np.float32),
                "att": np.ascontiguousarray(att_feats[sl]).astype(np.float32),
                "hT": np.ascontiguousarray(h[sl].T).astype(np.float32),
                "WT": WT,
                "wab": wab,
                "bias8": bias8,
                "ident": ident,
            }
        )
    return maps


def kernel(h, att_feats, p_att_feats, W_h2att, b_h2att, w_alpha, b_alpha):
    global _NC_CACHE
    h = np.asarray(h)
    att_feats = np.asarray(att_feats)
    p_att_feats = np.asarray(p_att_feats)
    W_h2att = np.asarray(W_h2att)
    b_h2att = np.asarray(b_h2att)
    w_alpha = np.asarray(w_alpha)
    if _NC_CACHE is None:
        _NC_CACHE = build_kernel()
    nc = _NC_CACHE
    maps = _in_maps(h, att_feats, p_att_feats, W_h2att, b_h2att, w_alpha)
    res = run_bass_kernel_spmd(nc, maps, core_ids=list(range(8)))
    out = np.concatenate([res.results[c]["out"] for c in range(8)], axis=0)
    return out.astype(np.float32)
